# revision 17
# baseline (speedup 1.0000x reference)
"""DyGraphTransformer forward on 8 trn2 NeuronCores (Bass/Tile).

Sequence-parallel: each core owns 64 of the 512 rows.  Per layer the
post-LN1 activations yT (bf16, 32 KB) are AllGathered; every core then
computes the full K^T / V locally from the gathered yT (trades a little
PE work for a smaller, earlier collective and no k/v unpack shuffles).
The Graphormer attention bias is computed once per core for its 64 query
rows via one-hot matmul gathers of two tiny projected tables, stored as
exp(bias) and folded into softmax multiplicatively (scores are small, so
softmax safely skips the max-subtraction; normalization happens after
A@V via a ones column).

v3: single-op one-hot compare (broadcast comparand), triple-buffered
one-hot tiles, packed const/weight/bias DMAs, per-head bias reorient,
layer-0 LN/AG emitted before the bias build, hidden-major FFN.
"""

import sys

sys.path.insert(0, "/opt/trn_rl_repo")

import contextlib

import numpy as np

import concourse.bacc as bacc
import concourse.bass as bass
import concourse.tile as tile
from concourse import mybir
from concourse.bass_utils import run_bass_kernel_spmd

# model dims
N, F, H, NH, L, W = 512, 256, 256, 8, 6, 2
DK = H // NH                 # 32
EEN_W = 32                   # edge_encode table entries
EDN = 128                    # edge_dist table entries
NC = 8                       # cores
T = N // NC                  # 64 tokens per core
NJT = N // 128               # 4 j-tiles
LN_EPS = 1e-5
SCALE = DK ** -0.5

F32 = mybir.dt.float32
BF16 = mybir.dt.bfloat16
I32 = mybir.dt.int32
AL = mybir.AluOpType
AF = mybir.ActivationFunctionType

GC1 = 0.7978845608028654     # sqrt(2/pi)
GC2 = GC1 * 0.044715

CC_WORDS = 8192              # f32 words holding yT bf16 [128,128]

_CACHE = {}

B_ORDER = ["bq", "bk", "b1", "ln1_s", "ln1_b", "ln2_s", "ln2_b"]
W_ORDER = ["Wq", "Wk", "Wv", "Wo", "W1", "W2"]


def _bcast_row(dram_ap, p):
    """DRAM AP -> broadcast AP with p partitions."""
    return bass.AP(tensor=dram_ap.tensor, offset=dram_ap.offset,
                   ap=[[0, p]] + [list(x) for x in dram_ap.ap])


def _bcol(col_ap, n):
    """SBUF column AP [128,1] -> free-broadcast AP [128, n]."""
    return bass.AP(tensor=col_ap.tensor, offset=col_ap.offset,
                   ap=[list(col_ap.ap[0])] + [[0, n]])


def build(tt_eq=False):
    nc = bacc.Bacc("TRN2", target_bir_lowering=False, debug=False,
                   num_devices=NC)

    # ---------------- DRAM I/O ----------------
    xw_in = nc.dram_tensor("xw", [128, 2, T + H], F32, kind="ExternalInput")
    bfeat_in = nc.dram_tensor("b_feat", [H], F32, kind="ExternalInput")
    cm_in = nc.dram_tensor("cm", [128, 132], F32, kind="ExternalInput")
    icmp_in = nc.dram_tensor("icmp", [128, 6], BF16, kind="ExternalInput")
    e_in = nc.dram_tensor("edge_emb", [EEN_W, H], F32, kind="ExternalInput")
    eT_in = nc.dram_tensor("edge_embT", [128, 2, EEN_W], F32, kind="ExternalInput")
    ed_in = nc.dram_tensor("edge_dist_emb", [EDN, H], F32, kind="ExternalInput")
    edT_in = nc.dram_tensor("edge_dist_embT", [128, 2, EDN], F32, kind="ExternalInput")
    wee_in = nc.dram_tensor("w_ee", [128, 2, NH], F32, kind="ExternalInput")
    wed_in = nc.dram_tensor("w_ed", [128, 2, NH], F32, kind="ExternalInput")
    wall_in = nc.dram_tensor("wall", [128, 6, L, 2, H], BF16, kind="ExternalInput")
    ball_in = nc.dram_tensor("ball", [128, L, 7, 2], F32, kind="ExternalInput")
    bmisc_in = nc.dram_tensor("bmisc", [L, 3, H], F32, kind="ExternalInput")
    t1a_in = nc.dram_tensor("t1a", [128, 4096], BF16, kind="ExternalInput")
    t1b_in = nc.dram_tensor("t1b", [128, 4096], BF16, kind="ExternalInput")
    t2_in = nc.dram_tensor("t2", [128, 8192], BF16, kind="ExternalInput")

    identb_in = nc.dram_tensor("identb", [128, 128], BF16, kind="ExternalInput")
    out_t = nc.dram_tensor("out", [T, H], F32, kind="ExternalOutput")
    p1_d = nc.dram_tensor("p1_d", [EEN_W, NH], BF16)
    p2_d = nc.dram_tensor("p2_d", [EDN, NH], BF16)

    cc_ins = [nc.dram_tensor(f"cc_in{i}", [CC_WORDS], F32) for i in range(L)]
    cc_outs = [nc.dram_tensor(f"cc_out{i}", [NC, CC_WORDS], F32,
                              addr_space="Shared") for i in range(L)]

    with tile.TileContext(nc) as tc:
        ctx = contextlib.ExitStack()
        with ctx:
            const = ctx.enter_context(tc.tile_pool(name="const", bufs=1))
            wpool = ctx.enter_context(tc.tile_pool(name="weights", bufs=1))
            small = ctx.enter_context(tc.tile_pool(name="small", bufs=2))
            work = ctx.enter_context(tc.tile_pool(name="work", bufs=2))
            psMM = ctx.enter_context(tc.tile_pool(name="psMM", bufs=2, space="PSUM"))
            psSC = ctx.enter_context(tc.tile_pool(name="psSC", bufs=2, space="PSUM"))
            psO = ctx.enter_context(tc.tile_pool(name="psO", bufs=2, space="PSUM"))

            # ---------------- constants (packed) ----------------
            cm = const.tile([128, 132], F32)
            nc.sync.dma_start(out=cm, in_=cm_in[:, :])
            ident = cm[:, 0:128]
            iota16f = cm[:, 128:129]
            iota32f = cm[:, 129:130]
            bee32 = cm[:, 130:131]
            bed32 = cm[:, 131:132]
            bt_tiles = {}
            for (nat_in, tT_in, wp_in, n_e, tagp) in [
                    (e_in, eT_in, wee_in, EEN_W, "t1"),
                    (ed_in, edT_in, wed_in, EDN, "t2")]:
                emb = const.tile([128, H], F32, tag=tagp + "nat")
                nc.sync.dma_start(out=emb[:n_e], in_=nat_in[:, :])
                embT = const.tile([128, 2, n_e], F32, tag=tagp + "T")
                nc.sync.dma_start(out=embT, in_=tT_in[:, :, :])
                wp = const.tile([128, 2, NH], F32, tag=tagp + "w")
                nc.sync.dma_start(out=wp, in_=wp_in[:, :, :])
                bt_tiles[tagp] = (emb, embT, wp)
            icmp = const.tile([128, 6], BF16)
            nc.scalar.dma_start(out=icmp, in_=icmp_in[:, :])
            identb = const.tile([128, 128], BF16)
            nc.scalar.dma_start(out=identb, in_=identb_in[:, :])
            magic = const.tile([128, 1], I32)
            nc.vector.memset(magic, 0x5F3759DF)
            bsum32 = const.tile([128, 1], F32)
            nc.vector.tensor_tensor(out=bsum32[:32], in0=bee32[:32],
                                    in1=bed32[:32], op=AL.add)

            # bulk inputs stream on the scalar ring; the sync ring stays
            # free for the small latency-critical table DMAs
            t1a_idx = const.tile([128, 4096], BF16, tag="t1a_idx")
            nc.scalar.dma_start(out=t1a_idx, in_=t1a_in[:, :])
            t1b_idx = const.tile([128, 4096], BF16, tag="t1b_idx")
            nc.scalar.dma_start(out=t1b_idx, in_=t1b_in[:, :])
            xw = const.tile([128, 2, T + H], F32)
            nc.scalar.dma_start(out=xw, in_=xw_in[:, :, :])
            t2_idx = const.tile([128, 8192], BF16, tag="t2_idx")
            nc.scalar.dma_start(out=t2_idx, in_=t2_in[:, :])
            wall = wpool.tile([128, 6, L, 2, H], BF16, tag="wall")
            nc.scalar.dma_start(out=wall, in_=wall_in[:, :, :, :, :])
            bfeat_r = const.tile([128, H], F32)
            nc.scalar.dma_start(out=bfeat_r[:T], in_=_bcast_row(bfeat_in.ap(), T))
            wsb = {n: wall[:, i] for i, n in enumerate(W_ORDER)}
            ball = wpool.tile([128, L, 7, 2], F32, tag="ball")
            nc.sync.dma_start(out=ball, in_=ball_in[:, :, :, :])
            bsb = {n: ball[:, :, i] for i, n in enumerate(B_ORDER)}
            bq_sc = wpool.tile([128, L, 2], F32, tag="b_bqsc")
            nc.vector.tensor_scalar(
                out=bq_sc, in0=bsb["bq"],
                scalar1=SCALE, scalar2=None, op0=AL.mult)

            # ---------------- helpers ----------------
            def rsqrt_col(u_ap, p, tagp, iters=1):
                """rsqrt of f32 column [p,1] via bit trick + Newton on DVE."""
                ki = small.tile([128, 1], I32, tag=tagp + "ki")
                nc.vector.tensor_scalar(out=ki[:p], in0=u_ap.bitcast(I32),
                                        scalar1=1, scalar2=None,
                                        op0=AL.logical_shift_right)
                z = small.tile([128, 1], F32, tag=tagp + "z")
                nc.vector.tensor_tensor(out=z[:p].bitcast(I32), in0=magic[:p],
                                        in1=ki[:p], op=AL.subtract)
                t = small.tile([128, 1], F32, tag=tagp + "t")
                for _ in range(iters):
                    nc.vector.tensor_scalar(out=t[:p], in0=z[:p], scalar1=z[:p],
                                            scalar2=u_ap, op0=AL.mult, op1=AL.mult)
                    nc.vector.tensor_scalar(out=t[:p], in0=t[:p], scalar1=-0.5,
                                            scalar2=1.5, op0=AL.mult, op1=AL.add)
                    nc.vector.tensor_tensor(out=z[:p], in0=z[:p], in1=t[:p],
                                            op=AL.mult)
                return z

            def layernorm_stats(h_ap, tagp):
                stats = small.tile([128, 6], F32, tag=tagp + "st")
                nc.vector.bn_stats(out=stats[:T], in_=h_ap)
                mv = small.tile([128, 2], F32, tag=tagp + "mv")
                nc.vector.bn_aggr(out=mv[:T], in_=stats[:T])
                u = small.tile([128, 1], F32, tag=tagp + "u")
                nc.vector.tensor_scalar(out=u[:T], in0=mv[:T, 1:2],
                                        scalar1=LN_EPS, scalar2=None, op0=AL.add)
                rstd = rsqrt_col(u[:T], T, tagp)
                return mv, rstd

            # =====================================================
            # h0 = x @ Wfeat + b  (token-major [64,256], fp32)
            # =====================================================
            h_sb = const.tile([128, H], F32, tag="resid")
            h_ps = psMM.tile([64, H], F32, tag="mm")
            for a in range(2):
                nc.tensor.matmul(h_ps, xw[:, a, 0:T], xw[:, a, T:T + H],
                                 start=(a == 0), stop=(a == 1))
            nc.vector.tensor_tensor(out=h_sb[:T], in0=h_ps, in1=bfeat_r[:T],
                                    op=AL.add)

            # =====================================================
            # per-layer: LN1 -> yT -> AllGather(yT); K/V from gathered yT
            # =====================================================
            def emit_y_ag(l):
                """LN1, yT, stage+AllGather, local qT (overlaps the AG)."""
                mv, rstd = layernorm_stats(h_sb[:T], "ln1")
                y = work.tile([128, H], F32, tag="y1")
                nc.vector.tensor_scalar(out=y[:T], in0=h_sb[:T],
                                        scalar1=mv[:T, 0:1], scalar2=rstd[:T],
                                        op0=AL.subtract, op1=AL.mult)
                yT = work.tile([128, 2, T], BF16, tag="y1T")
                for a in range(2):
                    tp = psMM.tile([128, T], F32, tag="mm")
                    nc.tensor.transpose(tp, y[:T, 128 * a:128 * (a + 1)],
                                        ident[:T, :T])
                    nc.scalar.activation(yT[:, a], tp, AF.Identity,
                                         bias=bsb["ln1_b"][:, l, a:a + 1],
                                         scale=bsb["ln1_s"][:, l, a:a + 1])
                nc.sync.dma_start(
                    out=cc_ins[l].ap()[0:CC_WORDS].rearrange(
                        "(p f) -> p f", p=128).bitcast(BF16),
                    in_=yT.rearrange("p a t -> p (a t)"))
                nc.gpsimd.collective_compute(
                    "AllGather", AL.bypass,
                    replica_groups=[list(range(NC))],
                    ins=[cc_ins[l][:]], outs=[cc_outs[l][:, :]])
                # local q (hidden-major), overlaps the collective
                qT = work.tile([128, 2, T], BF16, tag="qT")
                for m in range(2):
                    pp = psMM.tile([128, T], F32, tag="mm")
                    for a in range(2):
                        nc.tensor.matmul(
                            pp, wsb["Wq"][:, l, a, 128 * m:128 * (m + 1)],
                            yT[:, a], start=(a == 0), stop=(a == 1))
                    nc.scalar.activation(qT[:, m], pp, AF.Identity,
                                         bias=bq_sc[:, l, m:m + 1], scale=SCALE)
                # bv/bo/b2 broadcast rows for this layer
                bm_r = work.tile([128, 3, H], F32, tag="bm_r")
                nc.scalar.dma_start(
                    out=bm_r,
                    in_=_bcast_row(bmisc_in.ap()[l].rearrange("a f -> a f"), 128))
                return qT, bm_r

            def emit_kv(l, bv_row):
                """Unpack gathered yT; full K^T and V tiles from it."""
                yTf = work.tile([128, 2, N], BF16, tag="yTf")
                nc.sync.dma_start(
                    out=yTf.rearrange("p a (c t) -> p a c t", c=NC),
                    in_=cc_outs[l].ap()[:, 0:CC_WORDS].bitcast(BF16).rearrange(
                        "c (p a t) -> p a c t", p=128, a=2))
                ktf = work.tile([128, 2, N], BF16, tag="ktf")
                for m in range(2):
                    kp = psMM.tile([128, N], F32, tag="mm")
                    for a in range(2):
                        nc.tensor.matmul(
                            kp, wsb["Wk"][:, l, a, 128 * m:128 * (m + 1)],
                            yTf[:, a], start=(a == 0), stop=(a == 1))
                    nc.scalar.activation(ktf[:, m], kp, AF.Identity,
                                         bias=bsb["bk"][:, l, m:m + 1])
                vtiles = work.tile([128, NJT, NH, 33], BF16, tag="vtiles")
                nc.vector.memset(vtiles[:, :, :, 32:33].rearrange(
                    "p a b c -> p (a b c)"), 1.0)
                for jt in range(NJT):
                    vp = psSC.tile([128, H], F32, tag="sc")
                    for a in range(2):
                        nc.tensor.matmul(
                            vp, yTf[:, a, 128 * jt:128 * (jt + 1)],
                            wsb["Wv"][:, l, a], start=(a == 0), stop=(a == 1))
                    nc.vector.tensor_tensor(
                        out=vtiles[:, jt, :, 0:32],
                        in0=vp.rearrange("p (h d) -> p h d", h=NH),
                        in1=bv_row.rearrange("p (h d) -> p h d", h=NH),
                        op=AL.add)
                return ktf, vtiles

            # =====================================================
            # bias build: P1 [32,8], P2 [128,8] (renormed, projected tables)
            # =====================================================
            bctx = contextlib.ExitStack()
            bb = bctx.enter_context(tc.tile_pool(name="biasbuild", bufs=1))
            ohp = bctx.enter_context(tc.tile_pool(name="ohp", bufs=3))
            psOH = bctx.enter_context(tc.tile_pool(name="psOH", bufs=2, space="PSUM"))

            def build_table(n_e, tagp):
                emb, embT, wp = bt_tiles[tagp]
                sq = bb.tile([128, H], F32, tag=tagp + "sq")
                nc.vector.tensor_tensor(out=sq[:n_e], in0=emb[:n_e],
                                        in1=emb[:n_e], op=AL.mult)
                s = bb.tile([128, 1], F32, tag=tagp + "s")
                nc.vector.tensor_reduce(out=s[:n_e], in_=sq[:n_e],
                                        axis=mybir.AxisListType.X, op=AL.add)
                rs = rsqrt_col(s[:n_e], n_e, tagp)
                nrm = bb.tile([128, 1], F32, tag=tagp + "n")
                nc.vector.tensor_scalar(out=nrm[:n_e], in0=s[:n_e],
                                        scalar1=rs[:n_e], scalar2=1e-7,
                                        op0=AL.mult, op1=AL.add)
                nc.vector.tensor_scalar(out=nrm[:n_e], in0=nrm[:n_e],
                                        scalar1=1.0, scalar2=None, op0=AL.max)
                f = bb.tile([128, 1], F32, tag=tagp + "f")
                nc.vector.reciprocal(out=f[:n_e], in_=nrm[:n_e])
                pT_ps = psMM.tile([NH, 512], F32, tag="mm")
                for a in range(2):
                    nc.tensor.matmul(pT_ps[:, :n_e], wp[:, a], embT[:, a],
                                     start=(a == 0), stop=(a == 1))
                pT_sb = bb.tile([NH, 512], F32, tag=tagp + "pTs")
                nc.scalar.activation(pT_sb[:, :n_e], pT_ps[:, :n_e], AF.Copy)
                p_ps = psMM.tile([128, NH], F32, tag="mm")
                nc.tensor.transpose(p_ps[:n_e], pT_sb[:NH, :n_e],
                                    ident[:NH, :NH])
                p_sb = bb.tile([128, NH], F32, tag=tagp + "ps")
                nc.vector.tensor_scalar(out=p_sb[:n_e], in0=p_ps[:n_e],
                                        scalar1=f[:n_e], scalar2=None,
                                        op0=AL.mult)
                p_bf = bb.tile([128, NH], BF16, tag=tagp + "pbf")
                nc.vector.tensor_copy(out=p_bf[:n_e], in_=p_sb[:n_e])
                return p_bf

            p1 = build_table(EEN_W, "t1")
            nc.sync.dma_start(out=p1_d[:, :], in_=p1[:EEN_W])
            # block-diagonal lhsT tables (bf16): one strided DMA per g-group
            # t1 tile layout [p=(g,e), q, h, g']; lhsT for pass q = [:, q]
            t1t = const.tile([128, 2, 8, 8], BF16, tag="t1lhs")
            nc.vector.memset(t1t.rearrange("p a b c -> p (a b c)"), 0.0)
            for g in range(8):
                for q in range(2):
                    eng = nc.sync if (g + q) % 2 == 0 else nc.scalar
                    eng.dma_start(
                        out=t1t[16 * g:16 * g + 16, q, :, g:g + 1].rearrange(
                            "p b c -> p (b c)"),
                        in_=p1_d.ap()[16 * q:16 * q + 16, :])
            t1_lhs = [t1t[:, q].rearrange("p a b -> p (a b)") for q in range(2)]
            p2 = build_table(EDN, "t2")
            nc.sync.dma_start(out=p2_d[:, :], in_=p2[:EDN])
            t2t = const.tile([128, 4, 8, 4], BF16, tag="t2lhs")
            nc.vector.memset(t2t.rearrange("p a b c -> p (a b c)"), 0.0)
            for g in range(4):
                for q in range(4):
                    eng = nc.sync if (g + q) % 2 == 0 else nc.scalar
                    eng.dma_start(
                        out=t2t[32 * g:32 * g + 32, q, :, g:g + 1].rearrange(
                            "p b c -> p (b c)"),
                        in_=p2_d.ap()[32 * q:32 * q + 32, :])
            t2_lhs = [t2t[:, q].rearrange("p a b -> p (a b)") for q in range(4)]

            # layer-0 LN/AllGather: CC trigger sits behind the lhs DMAs on
            # the gpsimd queue, so it fires as soon as staging lands.
            qkv0 = emit_y_ag(0)

            # one-hot gathers -> head-major exp tables (bf16)
            t1a_hm = bb.tile([64, 4096], BF16, tag="t1a_hm")
            t1b_hm = bb.tile([64, 4096], BF16, tag="t1b_hm")
            t2_hm = bb.tile([32, 8192], BF16, tag="t2_hm")

            def onehot_gather(idx_tile, lhs_list, cmp0, n_sub, iota, ncols,
                              out_hm, mrows, scale, bias_ap):
                npass = len(lhs_list)
                for ch in range(ncols // 512):
                    ps = psOH.tile([64, 512], F32, tag="oh")
                    for q in range(npass):
                        oh = ohp.tile([128, 512], BF16, tag="ohc")
                        if tt_eq:
                            nc.vector.tensor_tensor(
                                out=oh, in0=idx_tile[:, 512 * ch:512 * (ch + 1)],
                                in1=_bcol(icmp[:, cmp0 + q:cmp0 + q + 1], 512),
                                op=AL.is_equal)
                        else:
                            nc.vector.tensor_scalar(
                                out=oh, in0=idx_tile[:, 512 * ch:512 * (ch + 1)],
                                scalar1=float(n_sub * q), scalar2=iota,
                                op0=AL.subtract, op1=AL.is_equal)
                        nc.tensor.matmul(ps[:mrows], lhs_list[q], oh,
                                         start=(q == 0), stop=(q == npass - 1))
                    nc.scalar.activation(
                        out_hm[:, 512 * ch:512 * (ch + 1)], ps[:mrows],
                        AF.Identity,
                        bias=bias_ap if bias_ap is not None else 0.0,
                        scale=scale)

            onehot_gather(t1a_idx, t1_lhs, 0, 16, iota16f, 4096, t1a_hm, 64,
                          0.5, None)
            onehot_gather(t1b_idx, t1_lhs, 0, 16, iota16f, 4096, t1b_hm, 64,
                          0.5, None)
            onehot_gather(t2_idx, t2_lhs, 2, 32, iota32f, 8192, t2_hm, 32,
                          1.0, bsum32[:32])

            # reorient to [j, i] per head; eb = t2 * t1a * t1b (per-head)
            eb = const.tile([128, NH, NJT, T], BF16, tag="eb")
            ebta = bb.tile([128, NH, NJT, T], BF16, tag="ebta")
            ebtb = bb.tile([128, NH, NJT, T], BF16, tag="ebtb")
            for h in range(NH):
                nc.sync.dma_start(
                    out=eb[:, h].rearrange("p j t -> p (j t)"),
                    in_=t2_hm[4 * h:4 * h + 4].rearrange(
                        "g (jj r) -> g jj r", jj=32, r=NJT * T))
                nc.scalar.dma_start(
                    out=ebta[:, h].rearrange("p j t -> p (j t)"),
                    in_=t1a_hm[8 * h:8 * h + 8].rearrange(
                        "g (jj r) -> g jj r", jj=16, r=NJT * T))
                nc.gpsimd.dma_start(
                    out=ebtb[:, h].rearrange("p j t -> p (j t)"),
                    in_=t1b_hm[8 * h:8 * h + 8].rearrange(
                        "g (jj r) -> g jj r", jj=16, r=NJT * T))
                nc.vector.tensor_tensor(
                    out=ebta[:, h].rearrange("p j t -> p (j t)"),
                    in0=ebta[:, h].rearrange("p j t -> p (j t)"),
                    in1=ebtb[:, h].rearrange("p j t -> p (j t)"), op=AL.add)
                nc.vector.tensor_tensor(
                    out=eb[:, h].rearrange("p j t -> p (j t)"),
                    in0=eb[:, h].rearrange("p j t -> p (j t)"),
                    in1=ebta[:, h].rearrange("p j t -> p (j t)"), op=AL.add)

            bctx.close()

            # =====================================================
            # layers
            # =====================================================
            for l in range(L):
                qT, bm_r = qkv0 if l == 0 else emit_y_ag(l)
                ktf, vtiles = emit_kv(l, bm_r[:, 0])
                hbo = work.tile([128, H], F32, tag="hbo")
                nc.vector.tensor_tensor(out=hbo[:T], in0=h_sb[:T],
                                        in1=bm_r[:T, 1], op=AL.add)

                # scores^T per head: bias seeded by identity matmul, QK
                # accumulates on top, single exp evict to probabilities
                probs2 = work.tile([128, NH, NJT, T], BF16, tag="probs2")
                o_ps = psO.tile([64, NH, 33], F32, tag="o")
                o_sb = work.tile([64, H], F32, tag="o_sb")
                oT = work.tile([128, 2, T], BF16, tag="oT")
                for h in range(NH):
                    bk_ps = psSC.tile([128, NJT, T], F32, tag="sc")
                    nc.tensor.matmul(
                        bk_ps.rearrange("p j t -> p (j t)"), identb,
                        eb[:, h].rearrange("p j t -> p (j t)"),
                        start=True, stop=False)
                    for jt in range(NJT):
                        nc.tensor.matmul(
                            bk_ps[:, jt],
                            ktf[32 * (h % 4):32 * (h % 4) + 32, h // 4,
                                128 * jt:128 * (jt + 1)],
                            qT[32 * (h % 4):32 * (h % 4) + 32, h // 4],
                            start=False, stop=(jt == NJT - 1),
                            tile_position=(32 * (h % 4), 0),
                            skip_group_check=True)
                    nc.scalar.activation(
                        probs2[:, h].rearrange("p j t -> p (j t)"),
                        bk_ps.rearrange("p j t -> p (j t)"), AF.Exp)
                    for jt in range(NJT):
                        nc.tensor.matmul(o_ps[:, h], probs2[:, h, jt],
                                         vtiles[:, jt, h],
                                         start=(jt == 0), stop=(jt == NJT - 1))
                    rec = small.tile([64, 1], F32, tag="rec")
                    nc.vector.reciprocal(out=rec, in_=o_ps[:, h, 32:33])
                    nc.vector.tensor_scalar(
                        out=o_sb[:, 32 * h:32 * (h + 1)], in0=o_ps[:, h, 0:32],
                        scalar1=rec, scalar2=None, op0=AL.mult)
                    if h % 4 == 3:
                        a = h // 4
                        tp = psMM.tile([128, T], F32, tag="mm")
                        nc.tensor.transpose(tp, o_sb[:, 128 * a:128 * (a + 1)],
                                            ident[:T, :T])
                        nc.scalar.activation(oT[:, a], tp, AF.Copy)

                # h += o @ Wo + bo
                at_ps = psMM.tile([64, H], F32, tag="mm")
                for a in range(2):
                    nc.tensor.matmul(at_ps, oT[:, a], wsb["Wo"][:, l, a],
                                     start=(a == 0), stop=(a == 1))
                nc.vector.tensor_tensor(out=h_sb[:T], in0=hbo[:T], in1=at_ps,
                                        op=AL.add)
                hb2 = work.tile([128, H], F32, tag="hb2")
                nc.vector.tensor_tensor(out=hb2[:T], in0=h_sb[:T],
                                        in1=bm_r[:T, 2], op=AL.add)

                # LN2 + FFN (hidden-major z; gelu on zT, no g transposes)
                mv2, rstd2 = layernorm_stats(h_sb[:T], "ln2")
                y2 = work.tile([128, H], F32, tag="y2")
                nc.vector.tensor_scalar(out=y2[:T], in0=h_sb[:T],
                                        scalar1=mv2[:T, 0:1], scalar2=rstd2[:T],
                                        op0=AL.subtract, op1=AL.mult)
                y2T = work.tile([128, 2, T], BF16, tag="y2T")
                for a in range(2):
                    tp = psMM.tile([128, T], F32, tag="mm")
                    nc.tensor.transpose(tp, y2[:T, 128 * a:128 * (a + 1)],
                                        ident[:T, :T])
                    nc.scalar.activation(y2T[:, a], tp, AF.Identity,
                                         bias=bsb["ln2_b"][:, l, a:a + 1],
                                         scale=bsb["ln2_s"][:, l, a:a + 1])
                zt = work.tile([128, 2, T], F32, tag="zt")
                for m in range(2):
                    zp = psMM.tile([128, T], F32, tag="mm")
                    for a in range(2):
                        nc.tensor.matmul(
                            zp, wsb["W1"][:, l, a, 128 * m:128 * (m + 1)],
                            y2T[:, a], start=(a == 0), stop=(a == 1))
                    nc.scalar.activation(zt[:, m], zp, AF.Identity,
                                         bias=bsb["b1"][:, l, m:m + 1])
                # gelu(zt), tanh approx (same ACT table set as exp)
                zh = work.tile([128, 2, T], F32, tag="zh")
                nc.vector.tensor_scalar(out=zh.rearrange("p a t -> p (a t)"),
                                        in0=zt.rearrange("p a t -> p (a t)"),
                                        scalar1=0.5, scalar2=None, op0=AL.mult)
                z2 = work.tile([128, 2, T], F32, tag="z2")
                nc.scalar.activation(z2.rearrange("p a t -> p (a t)"),
                                     zt.rearrange("p a t -> p (a t)"), AF.Square)
                gw = work.tile([128, 2, T], F32, tag="gw")
                nc.scalar.activation(gw.rearrange("p a t -> p (a t)"),
                                     z2.rearrange("p a t -> p (a t)"), AF.Copy,
                                     bias=GC1, scale=GC2)
                gu = work.tile([128, 2, T], F32, tag="gu")
                nc.vector.tensor_tensor(out=gu.rearrange("p a t -> p (a t)"),
                                        in0=gw.rearrange("p a t -> p (a t)"),
                                        in1=zt.rearrange("p a t -> p (a t)"),
                                        op=AL.mult)
                gt = work.tile([128, 2, T], F32, tag="gt")
                nc.scalar.activation(gt.rearrange("p a t -> p (a t)"),
                                     gu.rearrange("p a t -> p (a t)"), AF.Tanh)
                gT = work.tile([128, 2, T], BF16, tag="gT")
                nc.vector.scalar_tensor_tensor(
                    out=gT.rearrange("p a t -> p (a t)"),
                    in0=gt.rearrange("p a t -> p (a t)"), scalar=1.0,
                    in1=zh.rearrange("p a t -> p (a t)"),
                    op0=AL.add, op1=AL.mult)
                f_ps = psMM.tile([64, H], F32, tag="mm")
                for m in range(2):
                    nc.tensor.matmul(f_ps, gT[:, m], wsb["W2"][:, l, m],
                                     start=(m == 0), stop=(m == 1))
                nc.vector.tensor_tensor(out=h_sb[:T], in0=hb2[:T], in1=f_ps,
                                        op=AL.add)

            nc.sync.dma_start(out=out_t[:, :], in_=h_sb[:T])

    nc.compile()
    return nc


# ---------------- host marshalling (reshape/cast only) ----------------

def _prep_inputs(inputs):
    import ml_dtypes

    def f32(a):
        return np.ascontiguousarray(np.asarray(a, np.float32))

    x = f32(inputs["x"])
    ee = np.asarray(inputs["edge_encodes"]).astype(np.int64)
    ede = np.asarray(inputs["edge_dist_encodes"]).astype(np.int64)[:, :, 0]

    p = np.arange(128)
    cm = np.zeros((128, 132), np.float32)
    cm[:, 0:128] = np.eye(128, dtype=np.float32)
    cm[:, 128] = p % 16
    cm[:, 129] = p % 32
    # table-eviction partition p = 4h+g -> head h = p//4
    cm[:32, 130] = np.repeat(f32(inputs["b_ee"]), 4)
    cm[:32, 131] = np.repeat(f32(inputs["b_ed"]), 4)

    icmp = np.zeros((128, 6), np.float32)
    icmp[:, 0] = p % 16
    icmp[:, 1] = p % 16 + 16
    for q in range(4):
        icmp[:, 2 + q] = p % 32 + 32 * q

    wall = np.stack([f32(inputs[n]).reshape(L, 2, 128, H) for n in W_ORDER])
    wall = wall.transpose(3, 0, 1, 2, 4)            # [128, 6, L, 2, H]
    ball = np.stack([f32(inputs[n]) for n in B_ORDER], axis=1)
    ball = ball.reshape(L, 7, 2, 128).transpose(3, 0, 1, 2)  # [128, L, 7, 2]
    bmisc = np.stack([f32(inputs["bv"]), f32(inputs["bo"]),
                      f32(inputs["b2"])], axis=1)

    shared = {
        "b_feat": f32(inputs["b_feat"]),
        "cm": cm,
        "icmp": icmp.astype(ml_dtypes.bfloat16),
        "edge_emb": f32(inputs["edge_emb"]),
        "edge_embT": f32(np.asarray(inputs["edge_emb"], np.float32).T
                         .reshape(2, 128, EEN_W).transpose(1, 0, 2)),
        "edge_dist_emb": f32(inputs["edge_dist_emb"]),
        "edge_dist_embT": f32(np.asarray(inputs["edge_dist_emb"], np.float32).T
                              .reshape(2, 128, EDN).transpose(1, 0, 2)),
        "w_ee": f32(inputs["W_ee"]).reshape(2, 128, NH).transpose(1, 0, 2).copy(),
        "w_ed": f32(inputs["W_ed"]).reshape(2, 128, NH).transpose(1, 0, 2).copy(),
        "wall": np.ascontiguousarray(wall).astype(ml_dtypes.bfloat16),
        "identb": np.eye(128, dtype=np.float32).astype(ml_dtypes.bfloat16),
        "ball": np.ascontiguousarray(ball),
        "bmisc": np.ascontiguousarray(bmisc),
    }

    wfeat = f32(inputs["W_feat"]).reshape(2, 128, H).transpose(1, 0, 2)
    in_maps = []
    for c in range(NC):
        rows = slice(T * c, T * (c + 1))
        m = dict(shared)
        xT = x[rows].T.reshape(2, 128, T).transpose(1, 0, 2)
        xwm = np.concatenate([xT, wfeat], axis=2)
        m["xw"] = np.ascontiguousarray(xwm)
        # t2: [4 g, (jt jj i)] where j = jt*128 + g*32 + jj; pre-broadcast 32x
        e2 = ede[rows].T.astype(np.float32).reshape(NJT, 4, 32, T)
        m["t2"] = np.ascontiguousarray(np.repeat(
            e2.transpose(1, 2, 0, 3).reshape(4, 8192), 32,
            axis=0)).astype(ml_dtypes.bfloat16)
        # t1 per w: [8 g, (jj jt i)] where j = jt*128 + g*16 + jj; bcast 16x
        for w, nm in [(0, "t1a"), (1, "t1b")]:
            e1 = ee[rows, :, w].T.astype(np.float32).reshape(NJT, 8, 16, T)
            m[nm] = np.ascontiguousarray(np.repeat(
                e1.transpose(1, 2, 0, 3).reshape(8, 4096), 16,
                axis=0)).astype(ml_dtypes.bfloat16)
        in_maps.append(m)
    return in_maps


def kernel(**inputs):
    inputs.pop("_debug", False)
    trace = inputs.pop("_trace", False)
    tmpdir = inputs.pop("_tmpdir", None)
    key = "k"
    if key not in _CACHE:
        _CACHE[key] = build()
    nc = _CACHE[key]
    in_maps = _prep_inputs(inputs)
    res = run_bass_kernel_spmd(nc, in_maps, list(range(NC)),
                               trace=trace, tmpdir=tmpdir)
    kernel._last_res = res
    out = np.concatenate([res.results[c]["out"] for c in range(NC)], axis=0)
    return out
